# revision 31
# baseline (speedup 1.0000x reference)
"""Trainium2 Bass kernel for a dense transformer block (RMSNorm->MHA->res, RMSNorm->SwiGLU-FFN->res).

Sharding over 8 NeuronCores: fsdp=2 (batch) x tp=4 (attention heads / FFN hidden).
Core 4*b + t handles batch b with TP-rank t (heads 4t..4t+3, FFN hidden cols 2048t..2048(t+1)).

All on-device activations are feature-major ("transposed": [features, rows]) so every
matmul (out = lhsT.T @ rhs, contraction on the partition axis) chains without transposes.

fp8 (e4m3) DoubleRow matmuls: contraction pairs two adjacent 128-row K-chunks per
instruction. Used for q/k/v projections, PV+softmax-denominator, out-projection, and
all three FFN matmuls. The logits matmul (contraction = head_dim = 128, unpairable)
runs in bf16.

RMSNorm1 is applied POST-projection: q = (x@W)/rms is a per-row (per moving column)
scalar multiply, so the q/k/v matmuls consume host-prequantized x8 = 4x directly and
the norm chain (squares -> ms -> rsqrt -> broadcast) runs concurrently on DVE/ACT;
only the cheap per-head PSUM->SBUF scale-muls wait for it.

Power-of-2 scale plan (host folds rms scales into weights):
  x8 = 4x  wq8/wk8 = 64*w  wv8 = 256*w -> q/k psum = 256*rms*q -> *bcq(=1/(256 rms)) bf16
  v8 = v_psum*bcq = 4*v
  expt8 = 32*exp(logits/sqrt(d))  [exp bias=ln32]   ones8 = 0.5 -> denom psum = 16*sum
  pv psum = 128*pv -> ao8 = pv_psum * (1/denom_psum) = 8*ao
  wout8 = 64*wout -> proj psum = 512*attn -> bf16 rs1 partials (x2^-9)
  x2 = xte + RS(bf16) [f32 residual stream]; x2n8 = 4*rms2(x2) -> fp8 AllGather
  wg8/wu8 = 16*w -> gate/up psum = 64*z -> gelu(scale 2^-6) f32; act8 = gel*u_psum = 64*act
  wd8 = 256*wd -> down psum = 16384*y -> bf16 rs2 partials (x2^-14)
Collectives: RS1/RS2 bf16, AllGather fp8, mean-square AllReduce f32 (tiny).

Stage 1+2 stream in 512-row phases. FFN runs in 4 row-groups of 512 with RS2(g)
pipelined under group g+1; the stage-1 tail (stage3a(3), rsqn2(2,3), AG2(1)) is
emitted inside the FFN scope so it overlaps FFN groups 0-1.
"""

import numpy as np

EMBED = 2048
HEADS = 16
HEAD_DIM = 128
FF_HID = 8192
BATCH = 2
SEQ = 2048
EPS = 1e-6

N_CORES = 8
TP = 4
GROUPS = [[0, 1, 2, 3], [4, 5, 6, 7]]
H_LOC = HEADS // TP          # 4 heads per core
F_LOC = FF_HID // TP         # 2048 ffn-hidden per core
ROWS = SEQ                   # 2048 rows per batch
ROWS_T = ROWS // TP          # 512 rows per tp-rank
P = 128
NE = EMBED // P              # 16 embed chunks
NF = F_LOC // P              # 16 ffn chunks
NR = ROWS // P               # 16 row chunks
QB = 512                     # q-block / phase row count / matmul moving size
NQB = ROWS // QB             # 4 phases
RH = 1024                    # ffn row-half (AllGather granularity)
INV_SQRT_D = float(1.0 / np.sqrt(HEAD_DIM))
LN32 = float(np.log(32.0))

_NC_CACHE = {}


def build_kernel():
    import concourse.mybir as mybir
    import concourse.tile as tile
    from concourse import bacc

    f32 = mybir.dt.float32
    bf16 = mybir.dt.bfloat16
    f8 = mybir.dt.float8e4

    nc = bacc.Bacc("TRN2", target_bir_lowering=False, debug=False, num_devices=N_CORES)

    io = {}
    io["x8"] = nc.dram_tensor("x8", [EMBED, ROWS], f8, kind="ExternalInput").ap()
    io["xte"] = nc.dram_tensor("xte", [ROWS_T, ROWS], f32, kind="ExternalInput").ap()
    io["wq"] = nc.dram_tensor("wq", [EMBED, H_LOC, HEAD_DIM], f8, kind="ExternalInput").ap()
    io["wk"] = nc.dram_tensor("wk", [EMBED, H_LOC, HEAD_DIM], f8, kind="ExternalInput").ap()
    io["wv"] = nc.dram_tensor("wv", [EMBED, H_LOC * HEAD_DIM], f8, kind="ExternalInput").ap()
    io["wout"] = nc.dram_tensor("wout", [H_LOC * HEAD_DIM, EMBED], f8, kind="ExternalInput").ap()
    io["wg"] = nc.dram_tensor("wg", [EMBED, F_LOC], f8, kind="ExternalInput").ap()
    io["wu"] = nc.dram_tensor("wu", [EMBED, F_LOC], f8, kind="ExternalInput").ap()
    io["wd"] = nc.dram_tensor("wd", [F_LOC, EMBED], f8, kind="ExternalInput").ap()
    io["masks"] = nc.dram_tensor("masks", [P, QB + 3 * P], f8, kind="ExternalInput").ap()
    io["out"] = nc.dram_tensor("out", [ROWS_T, ROWS], f32, kind="ExternalOutput").ap()

    with tile.TileContext(nc) as tc:
        _emit(tc, nc, io)
    nc.compile()
    return nc


def _emit(tc, nc, io):
    from contextlib import ExitStack

    import concourse.mybir as mybir

    f32 = mybir.dt.float32
    f32r = mybir.dt.float32r
    bf16 = mybir.dt.bfloat16
    f8 = mybir.dt.float8e4
    AF = mybir.ActivationFunctionType
    DR = mybir.MatmulPerfMode.DoubleRow

    x8in, xte = io["x8"], io["xte"]
    wq, wk, wv = io["wq"], io["wk"], io["wv"]
    wout, wg, wu, wd, masks = io["wout"], io["wg"], io["wu"], io["wd"], io["masks"]
    out_ext = io["out"]

    def r3(ap2d, cols=None):
        """[(o p), q] dram view -> [p, o, q]; optionally slice columns first."""
        v = ap2d if cols is None else ap2d[:, cols]
        return v.rearrange("(o p) q -> p o q", p=P)

    ctx = ExitStack()
    with ctx:
        consts = ctx.enter_context(tc.tile_pool(name="consts", bufs=1))
        dram = ctx.enter_context(tc.tile_pool(name="dram", bufs=1, space="DRAM"))
        # cross-scope pool: FFN gate/up weights + x2 tiles that span stage1->FFN
        xpool = ctx.enter_context(tc.tile_pool(name="xpool", bufs=1))

        # pair-axis stride of dual-fp8 Ldweights must be 16B-aligned -> pad cols
        ones8_t = consts.tile([P, 2, 16], f8)
        nc.vector.memset(ones8_t[:], 0.5)
        ones8 = ones8_t[:, :, 0:1]
        eps16_sb = consts.tile([1, 1], f32)
        nc.vector.memset(eps16_sb[:], EPS / 16.0)
        epsq_sb = consts.tile([1, 1], f32)
        nc.vector.memset(epsq_sb[:], EPS * 65536.0)
        ln32_sb = consts.tile([P, 1], f32)
        nc.vector.memset(ln32_sb[:], LN32)

        wg_sb = xpool.tile([P, NE, F_LOC], f8)
        wu_sb = xpool.tile([P, NE, F_LOC], f8)
        # two rotating x2 slots (phase qb uses slot qb%2), alive across scopes
        x2q_t = xpool.tile([P, 2, H_LOC, QB], f32)

        rs1_in = dram.tile([NQB, EMBED, ROWS_T], bf16)
        rs1_out = dram.tile([NQB, ROWS_T, ROWS_T], bf16)
        ar_in = dram.tile([NQB, 1, ROWS_T], f32)
        ar_out = dram.tile([NQB, 1, ROWS_T], f32)
        ag2a_in = dram.tile([ROWS_T, RH], f8)
        ag2a_out = dram.tile([EMBED, RH], f8)
        ag2b_in = dram.tile([ROWS_T, RH], f8)
        ag2b_out = dram.tile([EMBED, RH], f8)
        rs2_in = dram.tile([NQB, EMBED, ROWS_T], bf16)
        rs2_out = dram.tile([NQB, ROWS_T, ROWS_T], bf16)
        # group 3's RS2 split in E-halves so the tail collective is half-size:
        # half h holds e-chunks with (e%4)//2 == h, i.e. rank rows h*256..h*256+255
        rs2_in3 = dram.tile([2, EMBED // 2, ROWS_T], bf16)
        rs2_out3 = dram.tile([2, ROWS_T // 2, ROWS_T], bf16)
        x2_scr = dram.tile([ROWS_T, ROWS], f32)

        # ---- helpers shared by stage1 and FFN scopes ----
        def emit_stage3a(qb, pool, pspool):
            """x2 = rs1_out + xte slice; mean-square partials -> tiny AllReduce."""
            cols = slice(qb * QB, (qb + 1) * QB)
            rs_sb = pool.tile([P, H_LOC, QB], bf16, tag="rs_sb", bufs=1,
                              name=f"rs_sb{qb}")
            nc.sync.dma_start(rs_sb[:], r3(rs1_out[qb]))
            x2q = x2q_t[:, qb % 2]
            nc.sync.dma_start(x2q, r3(xte, cols))
            ms_part = pspool.tile([1, QB], f32, tag="acc1", bufs=2, name=f"msp{qb}")
            for em in range(H_LOC):
                nc.vector.tensor_add(x2q[:, em, :], x2q[:, em, :], rs_sb[:, em, :])
            for em2 in range(H_LOC // 2):
                pr = slice(2 * em2, 2 * em2 + 2)
                sq8 = pool.tile([P, 2, QB], f8, tag="sq", bufs=2)
                nc.vector.tensor_mul(sq8[:], x2q[:, pr, :], x2q[:, pr, :])
                nc.tensor.matmul(ms_part[:], ones8[:], sq8[:],
                                 start=(em2 == 0), stop=(em2 == H_LOC // 2 - 1),
                                 perf_mode=DR)
            nc.sync.dma_start(r3(x2_scr, cols), x2q)
            ms_sb = pool.tile([1, QB], f32, tag="ms_sb", bufs=1)
            nc.vector.tensor_copy(ms_sb[:], ms_part[:])
            nc.sync.dma_start(ar_in[qb][:], ms_sb[:])
            nc.gpsimd.collective_compute(
                "AllReduce", mybir.AluOpType.add, replica_groups=GROUPS,
                ins=[ar_in[qb][:].opt()], outs=[ar_out[qb][:].opt()],
            )

        def emit_rsqn2(qb, pool):
            """4/rms2 of the AllReduced mean-square, normalize to fp8, ship to AG."""
            cols_half = slice((qb % 2) * QB, (qb % 2 + 1) * QB)
            arv = pool.tile([1, QB], f32, tag="arv", bufs=1)
            nc.sync.dma_start(arv[:], ar_out[qb][:])
            # ms_ar = 0.5*sum(x2^2) -> rms2/4 = sqrt(2*ms/(16E) + eps/16)
            rsq2 = pool.tile([1, QB], f32, tag="rsq2", bufs=1)
            nc.scalar.activation(rsq2[:], arv[:], AF.Sqrt, bias=eps16_sb[:],
                                 scale=1.0 / (8.0 * EMBED))
            rsq2_i = pool.tile([1, QB], f32, tag="rsq2i", bufs=1)
            nc.vector.reciprocal(rsq2_i[:], rsq2[:])
            bc2 = pool.tile([P, QB], f32, tag="bc", bufs=2)
            nc.gpsimd.partition_broadcast(bc2[:], rsq2_i[:])
            x2q = x2q_t[:, qb % 2]
            ag_in = ag2a_in if qb < 2 else ag2b_in
            ag3 = r3(ag_in, cols_half)
            for em in range(H_LOC):
                n2q = pool.tile([P, QB], f8, tag="n2q", bufs=2)
                nc.vector.tensor_mul(n2q[:], x2q[:, em, :], bc2[:])
                nc.sync.dma_start(ag3[:, em, :], n2q[:])

        def emit_ag2(half):
            i, o = (ag2a_in, ag2a_out) if half == 0 else (ag2b_in, ag2b_out)
            nc.gpsimd.collective_compute(
                "AllGather", mybir.AluOpType.bypass, replica_groups=GROUPS,
                ins=[i[:].opt()], outs=[o[:].opt()],
            )

        # ========== Stage 1+2 (fused phases): qkv + attention (+rms2 prep) ==========
        with (
            tc.tile_pool(name="kv_store", bufs=1) as kv_pool,
            tc.tile_pool(name="s1", bufs=2) as s1,
            tc.tile_pool(name="s1ps", bufs=2, space="PSUM") as s1ps,
        ):
            k_store = kv_pool.tile([P, H_LOC, ROWS], bf16)
            v8_store = kv_pool.tile([P, NR, H_LOC, HEAD_DIM], f8)
            mask_sb = kv_pool.tile([P, QB + 3 * P], f8)
            wq_sb = kv_pool.tile([P, NE, H_LOC * HEAD_DIM], f8)
            wk_sb = kv_pool.tile([P, NE, H_LOC * HEAD_DIM], f8)
            wv_sb = kv_pool.tile([P, NE, H_LOC * HEAD_DIM], f8)
            wo_sb = kv_pool.tile([P, H_LOC, EMBED], f8)

            xns = {}

            def emit_x_dma(qb):
                cols = slice(qb * QB, (qb + 1) * QB)
                x8 = s1.tile([P, NE, QB], f8, tag="x8", bufs=2, name=f"x8_{qb}")
                nc.sync.dma_start(x8[:], r3(x8in, cols))
                xns[(qb, "8")] = x8

            # phase-0 critical DMAs first, then weights, then the FFN prefetch
            emit_x_dma(0)
            nc.sync.dma_start(wq_sb[:], wq.rearrange("(o p) h d -> p o (h d)", p=P))
            nc.sync.dma_start(wk_sb[:], wk.rearrange("(o p) h d -> p o (h d)", p=P))
            nc.sync.dma_start(wv_sb[:], r3(wv))
            nc.sync.dma_start(mask_sb[:], masks[:])
            nc.sync.dma_start(wo_sb[:], r3(wout))
            nc.sync.dma_start(wg_sb[:], r3(wg))
            nc.sync.dma_start(wu_sb[:], r3(wu))

            def emit_sq_ms_step(qb, e2):
                """square + mean-accumulate for chunk-pair e2 of phase qb (fp8 DR)."""
                if e2 == 0:
                    ms = s1ps.tile([1, QB], f32, tag="acc1", bufs=2, name=f"ms{qb}")
                    xns[(qb, "ms")] = ms
                ms = xns[(qb, "ms")]
                sq8 = s1.tile([P, 2, QB], f8, tag="sq", bufs=2)
                pr = slice(2 * e2, 2 * e2 + 2)
                sl = xns[(qb, "8")][:, pr, :]
                # (0.25*4x)^2 = x^2 (max ~28, no fp8 overflow)
                nc.scalar.activation(sq8[:], sl, AF.Square, scale=0.25)
                nc.tensor.matmul(ms[:], ones8[:], sq8[:],
                                 start=(e2 == 0), stop=(e2 == NE // 2 - 1),
                                 perf_mode=DR)

            def emit_norm_tail(qb):
                """bcq = 1/(256*rms): ms_psum = 8*sum(x^2) via fp8 squares of 4x."""
                ms = xns.pop((qb, "ms"))
                # ms_psum = 0.5*sum(x^2) -> 256*rms = sqrt(131072*ms/E + 65536*eps)
                rsq = s1.tile([1, QB], f32, tag="rsq", bufs=1)
                nc.scalar.activation(rsq[:], ms[:], AF.Sqrt, bias=epsq_sb[:],
                                     scale=131072.0 / EMBED)
                rsq_i = s1.tile([1, QB], f32, tag="rsqi", bufs=1)
                nc.vector.reciprocal(rsq_i[:], rsq[:])
                bc = s1.tile([P, QB], f32, tag="bc", bufs=2, name=f"bcq{qb}")
                nc.gpsimd.partition_broadcast(bc[:], rsq_i[:])
                xns[(qb, "bc")] = bc

            def emit_qkv_mm(qb):
                """q/k/v projections from host-quantized x8 (no norm dependency)."""
                x8 = xns[(qb, "8")]
                ps = {}
                for h in range(H_LOC):
                    hd = slice(h * HEAD_DIM, (h + 1) * HEAD_DIM)
                    for nm, w_sb in (("q", wq_sb), ("k", wk_sb)):
                        p_ps = s1ps.tile([P, QB], f32, tag="proj", bufs=2,
                                         name=f"{nm}ps{qb}_{h}")
                        for e2 in range(NE // 2):
                            pr = slice(2 * e2, 2 * e2 + 2)
                            nc.tensor.matmul(p_ps[:], w_sb[:, pr, hd], x8[:, pr, :],
                                             start=(e2 == 0),
                                             stop=(e2 == NE // 2 - 1), perf_mode=DR)
                        ps[(nm, h)] = p_ps
                v_ps = [
                    s1ps.tile([P, H_LOC * HEAD_DIM], f32, tag=t, bufs=2,
                              name=f"v_ps{i}")
                    for i, t in enumerate(("lg", "lg", "pv", "pv"))
                ]
                for e2 in range(NE // 2):
                    pr = slice(2 * e2, 2 * e2 + 2)
                    for rc in range(QB // P):
                        nc.tensor.matmul(v_ps[rc][:],
                                         x8[:, pr, rc * P : (rc + 1) * P],
                                         wv_sb[:, pr, :],
                                         start=(e2 == 0), stop=(e2 == NE // 2 - 1),
                                         perf_mode=DR)
                ps["v"] = v_ps
                return ps

            def emit_qkv_scale(qb, ps):
                """apply bcq per moving column; frees PSUM slots in FIFO order."""
                bc = xns.pop((qb, "bc"))
                cols = slice(qb * QB, (qb + 1) * QB)
                q_ph = s1.tile([P, H_LOC, QB], bf16, tag="q_ph", bufs=1,
                               name=f"q{qb}")
                for h in range(H_LOC):
                    nc.vector.tensor_mul(q_ph[:, h, :], ps[("q", h)][:], bc[:])
                    nc.vector.tensor_mul(k_store[:, h, cols], ps[("k", h)][:], bc[:])
                for rc in range(QB // P):
                    rcg = qb * (QB // P) + rc
                    nc.vector.tensor_mul(
                        v8_store[:, rcg].rearrange("p h d -> p (h d)"),
                        ps["v"][rc][:], bc[:])
                return q_ph

            def emit_attention(qb, q_ph):
                ao8 = s1.tile([P, H_LOC, QB], f8, tag="ao_ph", bufs=1, name=f"ao{qb}")
                nk = (qb + 1) * (QB // P)
                for h in range(H_LOC):
                    pv_ps = s1ps.tile([P, QB], f32, tag="pv", bufs=2)
                    sum_ps = s1ps.tile([1, QB], f32, tag="acc1", bufs=2)
                    lg_tiles = {}
                    ex_tiles = {}

                    def emit_lg(kc):
                        lg = s1ps.tile([P, QB], f32, tag="lg", bufs=2)
                        nc.tensor.matmul(
                            lg[:], k_store[:, h, kc * P : (kc + 1) * P],
                            q_ph[:, h, :], start=True, stop=True)
                        lg_tiles[kc] = lg

                    emit_lg(0)
                    for kc in range(nk):
                        if kc + 1 < nk:
                            emit_lg(kc + 1)
                        lg = lg_tiles.pop(kc)
                        if kc % 2 == 0:
                            ex = s1.tile([P, 2, QB], f8, tag="expt", bufs=2)
                            ex_tiles[kc // 2] = ex
                        ex = ex_tiles[kc // 2]
                        nc.scalar.activation(ex[:, kc % 2, :], lg[:], AF.Exp,
                                             bias=ln32_sb[:], scale=INV_SQRT_D)
                        j = kc - qb * (QB // P)
                        if j >= 0:
                            off = (3 - j) * P
                            nc.vector.tensor_mul(ex[:, kc % 2, :], ex[:, kc % 2, :],
                                                 mask_sb[:, off : off + QB])
                        if kc % 2 == 1:
                            pc = kc // 2
                            first, last = pc == 0, pc == nk // 2 - 1
                            nc.tensor.matmul(pv_ps[:],
                                             v8_store[:, 2 * pc : 2 * pc + 2, h, :],
                                             ex[:], start=first, stop=last,
                                             perf_mode=DR)
                            nc.tensor.matmul(sum_ps[:], ones8[:], ex[:],
                                             start=first, stop=last, perf_mode=DR)
                    rec = s1.tile([1, QB], f32, tag="rec", bufs=2)
                    nc.vector.reciprocal(rec[:], sum_ps[:])
                    rbc = s1.tile([P, QB], f32, tag="rbc", bufs=2)
                    nc.gpsimd.partition_broadcast(rbc[:], rec[:])
                    nc.vector.tensor_mul(ao8[:, h, :], pv_ps[:], rbc[:])
                return ao8

            def emit_outproj_step(qb, e, ao8):
                """one e-chunk of the out-projection partials of phase qb."""
                pr_ps = s1ps.tile([P, QB], f32, tag="proj", bufs=2)
                ec = slice(e * P, (e + 1) * P)
                for c2 in range(H_LOC // 2):
                    pr = slice(2 * c2, 2 * c2 + 2)
                    nc.tensor.matmul(pr_ps[:], wo_sb[:, pr, ec], ao8[:, pr, :],
                                     start=(c2 == 0), stop=(c2 == H_LOC // 2 - 1),
                                     perf_mode=DR)
                pr_sb = s1.tile([P, QB], bf16, tag="pr_sb", bufs=2)
                nc.scalar.activation(pr_sb[:], pr_ps[:], AF.Copy, scale=1.0 / 512.0)
                nc.sync.dma_start(
                    r3(rs1_in[qb][e * P : (e + 1) * P, :]), pr_sb[:])

            def emit_rs1(qb):
                nc.gpsimd.collective_compute(
                    "ReduceScatter", mybir.AluOpType.add, replica_groups=GROUPS,
                    ins=[rs1_in[qb][:].opt()], outs=[rs1_out[qb][:].opt()],
                )

            # ---- phase schedule (collectives pipelined under later phases) ----
            # stage3a(j) runs at phase j+2 (RS1(j) has a full phase to finish so
            # its ms-matmuls never stall the PE); stage3a(2,3) + rsqn2(2,3) +
            # AG2(1) overlap the FFN's first groups.
            for e2 in range(NE // 2):
                emit_sq_ms_step(0, e2)
            emit_norm_tail(0)
            aos = {}
            for qb in range(NQB):
                ps = emit_qkv_mm(qb)
                q_ph = emit_qkv_scale(qb, ps)
                if qb + 1 < NQB:
                    emit_x_dma(qb + 1)
                aos[qb] = emit_attention(qb, q_ph)
                if qb >= 2:
                    emit_stage3a(qb - 2, s1, s1ps)
                if qb == 3:
                    emit_rsqn2(1, s1)
                    emit_ag2(0)
                if qb + 1 < NQB:
                    for e in range(NE):
                        if e < NE // 2:
                            emit_sq_ms_step(qb + 1, e)
                        emit_outproj_step(qb, e, aos[qb])
                    emit_norm_tail(qb + 1)
                else:
                    for e in range(NE):
                        emit_outproj_step(qb, e, aos[qb])
                if qb == 2:
                    emit_rsqn2(0, s1)
                emit_rs1(qb)

        # ========== Stage 5: FFN in 4 row-groups; stage-1 tail overlapped ==========
        with (
            tc.tile_pool(name="s5", bufs=1) as s5,
            tc.tile_pool(name="s5t", bufs=2) as s5t,
            tc.tile_pool(name="s5ps", bufs=2, space="PSUM") as s5ps,
        ):
            wd_sb = s5.tile([P, NF, EMBED], f8)

            def emit_gateup(g):
                ag_out_h = ag2a_out if g < 2 else ag2b_out
                gcols = slice((g % 2) * QB, (g % 2 + 1) * QB)
                n2_sb = s5t.tile([P, NE, QB], f8, tag="n2g", bufs=2)
                nc.sync.dma_start(n2_sb[:], r3(ag_out_h, gcols))
                if g == 0:
                    nc.sync.dma_start(wd_sb[:], r3(wd))
                act8 = s5t.tile([P, NF, QB], f8, tag="act", bufs=2)
                for f in range(NF):
                    fc = slice(f * P, (f + 1) * P)
                    g_ps = s5ps.tile([P, QB], f32, tag="gate", bufs=2)
                    for e2 in range(NE // 2):
                        pr = slice(2 * e2, 2 * e2 + 2)
                        nc.tensor.matmul(g_ps[:], wg_sb[:, pr, fc], n2_sb[:, pr, :],
                                         start=(e2 == 0), stop=(e2 == NE // 2 - 1),
                                         perf_mode=DR)
                    u_ps = s5ps.tile([P, QB], f32, tag="up", bufs=2)
                    for e2 in range(NE // 2):
                        pr = slice(2 * e2, 2 * e2 + 2)
                        nc.tensor.matmul(u_ps[:], wu_sb[:, pr, fc], n2_sb[:, pr, :],
                                         start=(e2 == 0), stop=(e2 == NE // 2 - 1),
                                         perf_mode=DR)
                    gel = s5t.tile([P, QB], f32, tag="gel", bufs=3)
                    nc.scalar.activation(gel[:], g_ps[:], AF.Gelu_apprx_tanh,
                                         scale=1.0 / 64.0)
                    nc.vector.tensor_mul(act8[:, f, :], gel[:], u_ps[:])
                return act8

            def emit_down(g, act8):
                order = (list(range(NE)) if g < 3 else
                         [e for e in range(NE) if e % 4 < 2]
                         + [e for e in range(NE) if e % 4 >= 2])
                for i, e in enumerate(order):
                    ec = slice(e * P, (e + 1) * P)
                    d_ps = s5ps.tile([P, QB], f32, tag="down", bufs=2)
                    for f2 in range(NF // 2):
                        pr = slice(2 * f2, 2 * f2 + 2)
                        nc.tensor.matmul(d_ps[:], wd_sb[:, pr, ec], act8[:, pr, :],
                                         start=(f2 == 0), stop=(f2 == NF // 2 - 1),
                                         perf_mode=DR)
                    d_sb = s5t.tile([P, QB], bf16, tag="dstage", bufs=3)
                    nc.scalar.activation(d_sb[:], d_ps[:], AF.Copy,
                                         scale=1.0 / 16384.0)
                    if g < 3:
                        dst = rs2_in[g][e * P : (e + 1) * P, :]
                    else:
                        t, j = e // 4, e % 4
                        row = t * 2 * P + (j % 2) * P
                        dst = rs2_in3[j // 2][row : row + P, :]
                    nc.sync.dma_start(r3(dst), d_sb[:])
                    if g == 3 and i == NE // 2 - 1:
                        nc.gpsimd.collective_compute(
                            "ReduceScatter", mybir.AluOpType.add,
                            replica_groups=GROUPS,
                            ins=[rs2_in3[0][:].opt()], outs=[rs2_out3[0][:].opt()],
                        )

            def emit_rs2(g):
                if g < 3:
                    i_ap, o_ap = rs2_in[g][:], rs2_out[g][:]
                else:
                    i_ap, o_ap = rs2_in3[1][:], rs2_out3[1][:]
                nc.gpsimd.collective_compute(
                    "ReduceScatter", mybir.AluOpType.add, replica_groups=GROUPS,
                    ins=[i_ap.opt()], outs=[o_ap.opt()],
                )

            def emit_stage6(g):
                cols = slice(g * QB, (g + 1) * QB)
                fsum = s5t.tile([P, H_LOC, QB], bf16, tag="fsum", bufs=2)
                if g < 3:
                    nc.sync.dma_start(fsum[:], r3(rs2_out[g]))
                else:
                    nc.sync.dma_start(fsum[:, 0:2, :], r3(rs2_out3[0]))
                    nc.sync.dma_start(fsum[:, 2:4, :], r3(rs2_out3[1]))
                fin = s5t.tile([P, H_LOC, QB], f32, tag="fin", bufs=2)
                nc.sync.dma_start(fin[:], r3(x2_scr, cols))
                nc.vector.tensor_add(fin[:], fin[:], fsum[:])
                nc.sync.dma_start(r3(out_ext, cols), fin[:])

            emit_stage3a(2, s5t, s5ps)   # RS1(2) finished during phase 3
            emit_rsqn2(2, s5t)
            act = emit_gateup(0)
            emit_stage3a(3, s5t, s5ps)   # waits RS1(3); overlaps group-0 compute
            emit_down(0, act)
            emit_rsqn2(3, s5t)           # AR(3) fired by stage3a(3)
            emit_ag2(1)                  # before RS2(0) so group 2 never waits
            emit_rs2(0)
            act = emit_gateup(1)
            emit_down(1, act)
            emit_stage6(0)
            emit_rs2(1)
            for g in (2, 3):
                act = emit_gateup(g)
                emit_down(g, act)
                emit_stage6(g - 1)
                emit_rs2(g)
            emit_stage6(3)


# ============================ host side ============================


def _prep_core_inputs(inputs):
    """Shard + transpose + fold rms scales into weights + quantize. 8 in_maps."""
    import ml_dtypes

    F8 = ml_dtypes.float8_e4m3
    BF = ml_dtypes.bfloat16

    x = np.asarray(inputs["x"], np.float32)          # [B, S, E]
    w_qkv = np.asarray(inputs["w_qkv"], np.float32)  # [E, H, 3D]
    w_out = np.asarray(inputs["w_out"], np.float32)  # [H, D, E]
    w_gate = np.asarray(inputs["w_gate"], np.float32)
    w_up = np.asarray(inputs["w_up"], np.float32)
    w_down = np.asarray(inputs["w_down"], np.float32)
    scale1 = np.asarray(inputs["scale1"], np.float32)
    scale2 = np.asarray(inputs["scale2"], np.float32)

    wqkv_s = w_qkv * scale1[:, None, None]
    wq_f = (wqkv_s[:, :, 0:HEAD_DIM] * 64.0).astype(F8)
    wk_f = (wqkv_s[:, :, HEAD_DIM : 2 * HEAD_DIM] * 64.0).astype(F8)
    wv_f = (wqkv_s[:, :, 2 * HEAD_DIM : 3 * HEAD_DIM] * 256.0).astype(F8)
    wout_f = (w_out.reshape(HEADS * HEAD_DIM, EMBED) * 64.0).astype(F8)
    wg_s = (w_gate * scale2[:, None] * 16.0).astype(F8)
    wu_s = (w_up * scale2[:, None] * 16.0).astype(F8)
    wd_s = (w_down * 256.0).astype(F8)

    kp = np.arange(P)[:, None]
    m = np.arange(QB + 3 * P)[None, :]
    masks = (m >= kp + 3 * P).astype(F8)  # mask_j = masks[:, (3-j)*128 : +512]

    in_maps = []
    for c in range(N_CORES):
        b, t = divmod(c, TP)
        hs = slice(H_LOC * t, H_LOC * (t + 1))
        fs = slice(F_LOC * t, F_LOC * (t + 1))
        xtb = np.ascontiguousarray(x[b].T)  # [E, S]
        in_maps.append(
            {
                "x8": (xtb * 4.0).astype(F8),
                "xte": np.ascontiguousarray(xtb[ROWS_T * t : ROWS_T * (t + 1), :]),
                "wq": np.ascontiguousarray(wq_f[:, hs, :]),
                "wk": np.ascontiguousarray(wk_f[:, hs, :]),
                "wv": np.ascontiguousarray(
                    wv_f[:, hs, :].reshape(EMBED, H_LOC * HEAD_DIM)),
                "wout": np.ascontiguousarray(
                    wout_f[H_LOC * HEAD_DIM * t : H_LOC * HEAD_DIM * (t + 1), :]
                ),
                "wg": np.ascontiguousarray(wg_s[:, fs]),
                "wu": np.ascontiguousarray(wu_s[:, fs]),
                "wd": np.ascontiguousarray(wd_s[fs, :]),
                "masks": np.ascontiguousarray(masks),
            }
        )
    return in_maps


def _install_profile_hook():
    import sys
    import types

    try:
        import antenv.axon_hooks  # noqa: F401

        return
    except ImportError:
        pass
    try:
        from trn_agent_boot.trn_boot import _ntff_profile_via_ctypes

        _hook = _ntff_profile_via_ctypes("/opt/axon/libaxon_pjrt.so")
        _mod = types.ModuleType("antenv.axon_hooks")
        _mod.get_axon_ntff_profile_hook = lambda: _hook
        sys.modules["antenv.axon_hooks"] = _mod
    except Exception:
        pass


def _run(nc, in_maps, trace=False, trace_cores=None):
    _install_profile_hook()
    from concourse.bass_utils import run_bass_kernel_spmd

    return run_bass_kernel_spmd(
        nc,
        in_maps,
        core_ids=list(range(N_CORES)),
        trace=trace,
        trace_cores=trace_cores,
    )


def kernel(**inputs):
    if "nc" not in _NC_CACHE:
        _NC_CACHE["nc"] = build_kernel()
    nc = _NC_CACHE["nc"]
    in_maps = _prep_core_inputs(inputs)
    res = _run(nc, in_maps)
    out = np.empty((BATCH, SEQ, EMBED), np.float32)
    for c in range(N_CORES):
        b, t = divmod(c, TP)
        out[b, :, ROWS_T * t : ROWS_T * (t + 1)] = res.results[c]["out"].T
    return out


if __name__ == "__main__":
    build_kernel()
    print("build ok")


# revision 34
# speedup vs baseline: 1.0133x; 1.0133x over previous
"""Trainium2 Bass kernel for a dense transformer block (RMSNorm->MHA->res, RMSNorm->SwiGLU-FFN->res).

Sharding over 8 NeuronCores: fsdp=2 (batch) x tp=4 (attention heads / FFN hidden).
Core 4*b + t handles batch b with TP-rank t (heads 4t..4t+3, FFN hidden cols 2048t..2048(t+1)).

All on-device activations are feature-major ("transposed": [features, rows]) so every
matmul (out = lhsT.T @ rhs, contraction on the partition axis) chains without transposes.

fp8 (e4m3) DoubleRow matmuls: contraction pairs two adjacent 128-row K-chunks per
instruction. Used for q/k/v projections, PV+softmax-denominator, out-projection, and
all three FFN matmuls. The logits matmul (contraction = head_dim = 128, unpairable)
runs in bf16.

RMSNorm1 is applied POST-projection: q = (x@W)/rms is a per-row (per moving column)
scalar multiply, so the q/k/v matmuls consume host-prequantized x8 = 4x directly and
the norm chain (squares -> ms -> rsqrt -> broadcast) runs concurrently on DVE/ACT;
only the cheap per-head PSUM->SBUF scale-muls wait for it.

Power-of-2 scale plan (host folds rms scales into weights):
  x8 = 4x  wq8/wk8 = 64*w  wv8 = 256*w -> q/k psum = 256*rms*q -> *bcq(=1/(256 rms)) bf16
  v8 = v_psum*bcq = 4*v
  expt8 = 32*exp(logits/sqrt(d))  [exp bias=ln32]   ones8 = 0.5 -> denom psum = 16*sum
  pv psum = 128*pv -> ao8 = pv_psum * (1/denom_psum) = 8*ao
  wout8 = 64*wout -> proj psum = 512*attn -> bf16 rs1 partials (x2^-9)
  x2 = xte + RS(bf16) [f32 residual stream]; x2n8 = 4*rms2(x2) -> fp8 AllGather
  wg8/wu8 = 16*w -> gate/up psum = 64*z -> gelu(scale 2^-6) f32; act8 = gel*u_psum = 64*act
  wd8 = 256*wd -> down psum = 16384*y -> bf16 rs2 partials (x2^-14)
Collectives: RS1/RS2 bf16, AllGather fp8, mean-square AllReduce f32 (tiny).

Stage 1+2 stream in 512-row phases. FFN runs in 4 row-groups of 512 with RS2(g)
pipelined under group g+1; the stage-1 tail (stage3a(3), rsqn2(2,3), AG2(1)) is
emitted inside the FFN scope so it overlaps FFN groups 0-1.
"""

import numpy as np

EMBED = 2048
HEADS = 16
HEAD_DIM = 128
FF_HID = 8192
BATCH = 2
SEQ = 2048
EPS = 1e-6

N_CORES = 8
TP = 4
GROUPS = [[0, 1, 2, 3], [4, 5, 6, 7]]
H_LOC = HEADS // TP          # 4 heads per core
F_LOC = FF_HID // TP         # 2048 ffn-hidden per core
ROWS = SEQ                   # 2048 rows per batch
ROWS_T = ROWS // TP          # 512 rows per tp-rank
P = 128
NE = EMBED // P              # 16 embed chunks
NF = F_LOC // P              # 16 ffn chunks
NR = ROWS // P               # 16 row chunks
QB = 512                     # q-block / phase row count / matmul moving size
NQB = ROWS // QB             # 4 phases
RH = 1024                    # ffn row-half (AllGather granularity)
INV_SQRT_D = float(1.0 / np.sqrt(HEAD_DIM))
LN32 = float(np.log(32.0))

_NC_CACHE = {}


def build_kernel():
    import concourse.mybir as mybir
    import concourse.tile as tile
    from concourse import bacc

    f32 = mybir.dt.float32
    bf16 = mybir.dt.bfloat16
    f8 = mybir.dt.float8e4

    nc = bacc.Bacc("TRN2", target_bir_lowering=False, debug=False, num_devices=N_CORES)

    io = {}
    io["x8"] = nc.dram_tensor("x8", [EMBED, ROWS], f8, kind="ExternalInput").ap()
    io["xte"] = nc.dram_tensor("xte", [ROWS_T, ROWS], f32, kind="ExternalInput").ap()
    io["wq"] = nc.dram_tensor("wq", [EMBED, H_LOC, HEAD_DIM], f8, kind="ExternalInput").ap()
    io["wk"] = nc.dram_tensor("wk", [EMBED, H_LOC, HEAD_DIM], f8, kind="ExternalInput").ap()
    io["wv"] = nc.dram_tensor("wv", [EMBED, H_LOC * HEAD_DIM], f8, kind="ExternalInput").ap()
    io["wout"] = nc.dram_tensor("wout", [H_LOC * HEAD_DIM, EMBED], f8, kind="ExternalInput").ap()
    io["wg"] = nc.dram_tensor("wg", [EMBED, F_LOC], f8, kind="ExternalInput").ap()
    io["wu"] = nc.dram_tensor("wu", [EMBED, F_LOC], f8, kind="ExternalInput").ap()
    io["wd"] = nc.dram_tensor("wd", [F_LOC, EMBED], f8, kind="ExternalInput").ap()
    io["masks"] = nc.dram_tensor("masks", [P, QB + 3 * P], f8, kind="ExternalInput").ap()
    io["out"] = nc.dram_tensor("out", [ROWS_T, ROWS], f32, kind="ExternalOutput").ap()

    with tile.TileContext(nc) as tc:
        _emit(tc, nc, io)
    nc.compile()
    return nc


def _emit(tc, nc, io):
    from contextlib import ExitStack

    import concourse.mybir as mybir

    f32 = mybir.dt.float32
    f32r = mybir.dt.float32r
    bf16 = mybir.dt.bfloat16
    f8 = mybir.dt.float8e4
    AF = mybir.ActivationFunctionType
    DR = mybir.MatmulPerfMode.DoubleRow

    x8in, xte = io["x8"], io["xte"]
    wq, wk, wv = io["wq"], io["wk"], io["wv"]
    wout, wg, wu, wd, masks = io["wout"], io["wg"], io["wu"], io["wd"], io["masks"]
    out_ext = io["out"]

    def r3(ap2d, cols=None):
        """[(o p), q] dram view -> [p, o, q]; optionally slice columns first."""
        v = ap2d if cols is None else ap2d[:, cols]
        return v.rearrange("(o p) q -> p o q", p=P)

    ctx = ExitStack()
    with ctx:
        consts = ctx.enter_context(tc.tile_pool(name="consts", bufs=1))
        dram = ctx.enter_context(tc.tile_pool(name="dram", bufs=1, space="DRAM"))
        # cross-scope pool: FFN gate/up weights + x2 tiles that span stage1->FFN
        xpool = ctx.enter_context(tc.tile_pool(name="xpool", bufs=1))

        # pair-axis stride of dual-fp8 Ldweights must be 16B-aligned -> pad cols
        ones8_t = consts.tile([P, 2, 16], f8)
        nc.vector.memset(ones8_t[:], 0.5)
        ones8 = ones8_t[:, :, 0:1]
        eps16_sb = consts.tile([1, 1], f32)
        nc.vector.memset(eps16_sb[:], EPS / 16.0)
        epsq_sb = consts.tile([1, 1], f32)
        nc.vector.memset(epsq_sb[:], EPS * 65536.0)
        ln32_sb = consts.tile([P, 1], f32)
        nc.vector.memset(ln32_sb[:], LN32)

        wg_sb = xpool.tile([P, NE, F_LOC], f8)
        wu_sb = xpool.tile([P, NE, F_LOC], f8)
        # two rotating x2 slots (phase qb uses slot qb%2), alive across scopes
        x2q_t = xpool.tile([P, 2, H_LOC, QB], f32)

        rs1_in = dram.tile([NQB, EMBED, ROWS_T], bf16)
        rs1_out = dram.tile([NQB, ROWS_T, ROWS_T], bf16)
        ar_in = dram.tile([NQB, 1, ROWS_T], f32)
        ar_out = dram.tile([NQB, 1, ROWS_T], f32)
        ag2a_in = dram.tile([ROWS_T, RH], f8)
        ag2a_out = dram.tile([EMBED, RH], f8)
        ag2b_in = dram.tile([ROWS_T, RH], f8)
        ag2b_out = dram.tile([EMBED, RH], f8)
        rs2_in = dram.tile([NQB, EMBED, ROWS_T], bf16)
        rs2_out = dram.tile([NQB, ROWS_T, ROWS_T], bf16)
        # group 3's RS2 split in E-halves so the tail collective is half-size:
        # half h holds e-chunks with (e%4)//2 == h, i.e. rank rows h*256..h*256+255
        rs2_in3 = dram.tile([2, EMBED // 2, ROWS_T], bf16)
        rs2_out3 = dram.tile([2, ROWS_T // 2, ROWS_T], bf16)
        x2_scr = dram.tile([ROWS_T, ROWS], f32)

        # ---- helpers shared by stage1 and FFN scopes ----
        def emit_stage3a(qb, pool, pspool):
            """x2 = rs1_out + xte slice; mean-square partials -> tiny AllReduce."""
            cols = slice(qb * QB, (qb + 1) * QB)
            rs_sb = pool.tile([P, H_LOC, QB], bf16, tag="rs_sb", bufs=1,
                              name=f"rs_sb{qb}")
            nc.sync.dma_start(rs_sb[:], r3(rs1_out[qb]))
            x2q = x2q_t[:, qb % 2]
            nc.sync.dma_start(x2q, r3(xte, cols))
            ms_part = pspool.tile([1, QB], f32, tag="acc1", bufs=2, name=f"msp{qb}")
            for em in range(H_LOC):
                nc.vector.tensor_add(x2q[:, em, :], x2q[:, em, :], rs_sb[:, em, :])
            for em2 in range(H_LOC // 2):
                pr = slice(2 * em2, 2 * em2 + 2)
                sq8 = pool.tile([P, 2, QB], f8, tag="sq", bufs=2)
                nc.vector.tensor_mul(sq8[:], x2q[:, pr, :], x2q[:, pr, :])
                nc.tensor.matmul(ms_part[:], ones8[:], sq8[:],
                                 start=(em2 == 0), stop=(em2 == H_LOC // 2 - 1),
                                 perf_mode=DR)
            nc.sync.dma_start(r3(x2_scr, cols), x2q)
            ms_sb = pool.tile([1, QB], f32, tag="ms_sb", bufs=1)
            nc.vector.tensor_copy(ms_sb[:], ms_part[:])
            nc.sync.dma_start(ar_in[qb][:], ms_sb[:])
            nc.gpsimd.collective_compute(
                "AllReduce", mybir.AluOpType.add, replica_groups=GROUPS,
                ins=[ar_in[qb][:].opt()], outs=[ar_out[qb][:].opt()],
            )

        def emit_rsqn2(qb, pool):
            """4/rms2 of the AllReduced mean-square, normalize to fp8, ship to AG."""
            cols_half = slice((qb % 2) * QB, (qb % 2 + 1) * QB)
            arv = pool.tile([1, QB], f32, tag="arv", bufs=1)
            nc.sync.dma_start(arv[:], ar_out[qb][:])
            # ms_ar = 0.5*sum(x2^2) -> rms2/4 = sqrt(2*ms/(16E) + eps/16)
            rsq2 = pool.tile([1, QB], f32, tag="rsq2", bufs=1)
            nc.scalar.activation(rsq2[:], arv[:], AF.Sqrt, bias=eps16_sb[:],
                                 scale=1.0 / (8.0 * EMBED))
            rsq2_i = pool.tile([1, QB], f32, tag="rsq2i", bufs=1)
            nc.vector.reciprocal(rsq2_i[:], rsq2[:])
            bc2 = pool.tile([P, QB], f32, tag="bc", bufs=2)
            nc.gpsimd.partition_broadcast(bc2[:], rsq2_i[:])
            x2q = x2q_t[:, qb % 2]
            ag_in = ag2a_in if qb < 2 else ag2b_in
            ag3 = r3(ag_in, cols_half)
            for em in range(H_LOC):
                n2q = pool.tile([P, QB], f8, tag="n2q", bufs=2)
                nc.vector.tensor_mul(n2q[:], x2q[:, em, :], bc2[:])
                nc.sync.dma_start(ag3[:, em, :], n2q[:])

        def emit_ag2(half):
            i, o = (ag2a_in, ag2a_out) if half == 0 else (ag2b_in, ag2b_out)
            nc.gpsimd.collective_compute(
                "AllGather", mybir.AluOpType.bypass, replica_groups=GROUPS,
                ins=[i[:].opt()], outs=[o[:].opt()],
            )

        # ========== Stage 1+2 (fused phases): qkv + attention (+rms2 prep) ==========
        with (
            tc.tile_pool(name="kv_store", bufs=1) as kv_pool,
            tc.tile_pool(name="s1", bufs=2) as s1,
            tc.tile_pool(name="s1ps", bufs=2, space="PSUM") as s1ps,
        ):
            k_store = kv_pool.tile([P, H_LOC, ROWS], bf16)
            v8_store = kv_pool.tile([P, NR, H_LOC, HEAD_DIM], f8)
            mask_sb = kv_pool.tile([P, QB + 3 * P], f8)
            wq_sb = kv_pool.tile([P, NE, H_LOC * HEAD_DIM], f8)
            wk_sb = kv_pool.tile([P, NE, H_LOC * HEAD_DIM], f8)
            wv_sb = kv_pool.tile([P, NE, H_LOC * HEAD_DIM], f8)
            wo_sb = kv_pool.tile([P, H_LOC, EMBED], f8)

            xns = {}

            def emit_x_dma(qb):
                cols = slice(qb * QB, (qb + 1) * QB)
                x8 = s1.tile([P, NE, QB], f8, tag="x8", bufs=2, name=f"x8_{qb}")
                nc.sync.dma_start(x8[:], r3(x8in, cols))
                xns[(qb, "8")] = x8

            # phase-0 critical DMAs first, then weights, then the FFN prefetch
            emit_x_dma(0)
            nc.sync.dma_start(wq_sb[:], wq.rearrange("(o p) h d -> p o (h d)", p=P))
            nc.sync.dma_start(wk_sb[:], wk.rearrange("(o p) h d -> p o (h d)", p=P))
            nc.sync.dma_start(wv_sb[:], r3(wv))
            nc.sync.dma_start(mask_sb[:], masks[:])
            nc.sync.dma_start(wo_sb[:], r3(wout))
            nc.sync.dma_start(wg_sb[:], r3(wg))
            nc.sync.dma_start(wu_sb[:], r3(wu))

            def emit_sq_ms_step(qb, e2):
                """square + mean-accumulate for chunk-pair e2 of phase qb (fp8 DR)."""
                if e2 == 0:
                    ms = s1ps.tile([1, QB], f32, tag="acc1", bufs=2, name=f"ms{qb}")
                    xns[(qb, "ms")] = ms
                ms = xns[(qb, "ms")]
                sq8 = s1.tile([P, 2, QB], f8, tag="sq", bufs=2)
                pr = slice(2 * e2, 2 * e2 + 2)
                sl = xns[(qb, "8")][:, pr, :]
                # (0.25*4x)^2 = x^2 (max ~28, no fp8 overflow)
                nc.scalar.activation(sq8[:], sl, AF.Square, scale=0.25)
                nc.tensor.matmul(ms[:], ones8[:], sq8[:],
                                 start=(e2 == 0), stop=(e2 == NE // 2 - 1),
                                 perf_mode=DR)

            def emit_norm_tail(qb):
                """bcq = 1/(256*rms): ms_psum = 8*sum(x^2) via fp8 squares of 4x."""
                ms = xns.pop((qb, "ms"))
                # ms_psum = 0.5*sum(x^2) -> 256*rms = sqrt(131072*ms/E + 65536*eps)
                rsq = s1.tile([1, QB], f32, tag="rsq", bufs=1)
                nc.scalar.activation(rsq[:], ms[:], AF.Sqrt, bias=epsq_sb[:],
                                     scale=131072.0 / EMBED)
                rsq_i = s1.tile([1, QB], f32, tag="rsqi", bufs=1)
                nc.vector.reciprocal(rsq_i[:], rsq[:])
                bc = s1.tile([P, QB], f32, tag="bc", bufs=2, name=f"bcq{qb}")
                nc.gpsimd.partition_broadcast(bc[:], rsq_i[:])
                xns[(qb, "bc")] = bc

            def emit_qkv_mm(qb):
                """q/k/v projections from host-quantized x8 (no norm dependency)."""
                x8 = xns[(qb, "8")]
                ps = {}
                for h in range(H_LOC):
                    hd = slice(h * HEAD_DIM, (h + 1) * HEAD_DIM)
                    for nm, w_sb in (("q", wq_sb), ("k", wk_sb)):
                        p_ps = s1ps.tile([P, QB], f32, tag="proj", bufs=2,
                                         name=f"{nm}ps{qb}_{h}")
                        for e2 in range(NE // 2):
                            pr = slice(2 * e2, 2 * e2 + 2)
                            nc.tensor.matmul(p_ps[:], w_sb[:, pr, hd], x8[:, pr, :],
                                             start=(e2 == 0),
                                             stop=(e2 == NE // 2 - 1), perf_mode=DR)
                        ps[(nm, h)] = p_ps
                v_ps = [
                    s1ps.tile([P, H_LOC * HEAD_DIM], f32, tag=t, bufs=2,
                              name=f"v_ps{i}")
                    for i, t in enumerate(("lg", "lg", "pv", "pv"))
                ]
                for e2 in range(NE // 2):
                    pr = slice(2 * e2, 2 * e2 + 2)
                    for rc in range(QB // P):
                        nc.tensor.matmul(v_ps[rc][:],
                                         x8[:, pr, rc * P : (rc + 1) * P],
                                         wv_sb[:, pr, :],
                                         start=(e2 == 0), stop=(e2 == NE // 2 - 1),
                                         perf_mode=DR)
                ps["v"] = v_ps
                return ps

            def emit_qkv_scale(qb, ps):
                """apply bcq per moving column; frees PSUM slots in FIFO order."""
                bc = xns.pop((qb, "bc"))
                cols = slice(qb * QB, (qb + 1) * QB)
                q_ph = s1.tile([P, H_LOC, QB], bf16, tag="q_ph", bufs=1,
                               name=f"q{qb}")
                for h in range(H_LOC):
                    nc.vector.tensor_mul(q_ph[:, h, :], ps[("q", h)][:], bc[:])
                    nc.vector.tensor_mul(k_store[:, h, cols], ps[("k", h)][:], bc[:])
                for rc in range(QB // P):
                    rcg = qb * (QB // P) + rc
                    nc.vector.tensor_mul(
                        v8_store[:, rcg].rearrange("p h d -> p (h d)"),
                        ps["v"][rc][:], bc[:])
                return q_ph

            def emit_attention(qb, q_ph):
                ao8 = s1.tile([P, H_LOC, QB], f8, tag="ao_ph", bufs=1, name=f"ao{qb}")
                nk = (qb + 1) * (QB // P)
                for h in range(H_LOC):
                    pv_ps = s1ps.tile([P, QB], f32, tag="pv", bufs=2)
                    sum_ps = s1ps.tile([1, QB], f32, tag="acc1", bufs=2)
                    lg_tiles = {}
                    ex_tiles = {}

                    def emit_lg(kc):
                        lg = s1ps.tile([P, QB], f32, tag="lg", bufs=2)
                        nc.tensor.matmul(
                            lg[:], k_store[:, h, kc * P : (kc + 1) * P],
                            q_ph[:, h, :], start=True, stop=True)
                        lg_tiles[kc] = lg

                    emit_lg(0)
                    for kc in range(nk):
                        if kc + 1 < nk:
                            emit_lg(kc + 1)
                        lg = lg_tiles.pop(kc)
                        if kc % 2 == 0:
                            ex = s1.tile([P, 2, QB], f8, tag="expt", bufs=2)
                            ex_tiles[kc // 2] = ex
                        ex = ex_tiles[kc // 2]
                        nc.scalar.activation(ex[:, kc % 2, :], lg[:], AF.Exp,
                                             bias=ln32_sb[:], scale=INV_SQRT_D)
                        j = kc - qb * (QB // P)
                        if j >= 0:
                            off = (3 - j) * P
                            nc.vector.tensor_mul(ex[:, kc % 2, :], ex[:, kc % 2, :],
                                                 mask_sb[:, off : off + QB])
                        if kc % 2 == 1:
                            pc = kc // 2
                            first, last = pc == 0, pc == nk // 2 - 1
                            nc.tensor.matmul(pv_ps[:],
                                             v8_store[:, 2 * pc : 2 * pc + 2, h, :],
                                             ex[:], start=first, stop=last,
                                             perf_mode=DR)
                            nc.tensor.matmul(sum_ps[:], ones8[:], ex[:],
                                             start=first, stop=last, perf_mode=DR)
                    rec = s1.tile([1, QB], f32, tag="rec", bufs=2)
                    nc.vector.reciprocal(rec[:], sum_ps[:])
                    rbc = s1.tile([P, QB], f32, tag="rbc", bufs=2)
                    nc.gpsimd.partition_broadcast(rbc[:], rec[:])
                    nc.vector.tensor_mul(ao8[:, h, :], pv_ps[:], rbc[:])
                return ao8

            def emit_outproj_step(qb, e, ao8):
                """one e-chunk of the out-projection partials of phase qb."""
                pr_ps = s1ps.tile([P, QB], f32, tag="proj", bufs=2)
                ec = slice(e * P, (e + 1) * P)
                for c2 in range(H_LOC // 2):
                    pr = slice(2 * c2, 2 * c2 + 2)
                    nc.tensor.matmul(pr_ps[:], wo_sb[:, pr, ec], ao8[:, pr, :],
                                     start=(c2 == 0), stop=(c2 == H_LOC // 2 - 1),
                                     perf_mode=DR)
                pr_sb = s1.tile([P, QB], bf16, tag="pr_sb", bufs=2)
                nc.scalar.activation(pr_sb[:], pr_ps[:], AF.Copy, scale=1.0 / 512.0)
                nc.sync.dma_start(
                    r3(rs1_in[qb][e * P : (e + 1) * P, :]), pr_sb[:])

            def emit_rs1(qb):
                nc.gpsimd.collective_compute(
                    "ReduceScatter", mybir.AluOpType.add, replica_groups=GROUPS,
                    ins=[rs1_in[qb][:].opt()], outs=[rs1_out[qb][:].opt()],
                )

            # ---- phase schedule (collectives pipelined under later phases) ----
            for e2 in range(NE // 2):
                emit_sq_ms_step(0, e2)
            emit_norm_tail(0)
            aos = {}
            for qb in range(NQB):
                ps = emit_qkv_mm(qb)
                q_ph = emit_qkv_scale(qb, ps)
                if qb + 1 < NQB:
                    emit_x_dma(qb + 1)
                aos[qb] = emit_attention(qb, q_ph)
                if qb == 2:
                    # RS1(0) finished a full phase ago -> no PE stall here
                    emit_stage3a(0, s1, s1ps)
                if qb == 3:
                    emit_rsqn2(1, s1)
                    emit_ag2(0)
                if qb + 1 < NQB:
                    for e in range(NE):
                        if e < NE // 2:
                            emit_sq_ms_step(qb + 1, e)
                        emit_outproj_step(qb, e, aos[qb])
                    emit_norm_tail(qb + 1)
                else:
                    for e in range(NE):
                        emit_outproj_step(qb, e, aos[qb])
                if qb == 2:
                    emit_rsqn2(0, s1)
                    emit_stage3a(1, s1, s1ps)
                if qb == 3:
                    emit_stage3a(2, s1, s1ps)
                emit_rs1(qb)

        # ========== Stage 5: FFN in 4 row-groups; stage-1 tail overlapped ==========
        with (
            tc.tile_pool(name="s5", bufs=1) as s5,
            tc.tile_pool(name="s5t", bufs=2) as s5t,
            tc.tile_pool(name="s5ps", bufs=2, space="PSUM") as s5ps,
        ):
            wd_sb = s5.tile([P, NF, EMBED], f8)

            def emit_gateup(g):
                ag_out_h = ag2a_out if g < 2 else ag2b_out
                gcols = slice((g % 2) * QB, (g % 2 + 1) * QB)
                n2_sb = s5t.tile([P, NE, QB], f8, tag="n2g", bufs=2)
                nc.sync.dma_start(n2_sb[:], r3(ag_out_h, gcols))
                if g == 0:
                    nc.sync.dma_start(wd_sb[:], r3(wd))
                act8 = s5t.tile([P, NF, QB], f8, tag="act", bufs=2)
                for f in range(NF):
                    fc = slice(f * P, (f + 1) * P)
                    g_ps = s5ps.tile([P, QB], f32, tag="gate", bufs=2)
                    for e2 in range(NE // 2):
                        pr = slice(2 * e2, 2 * e2 + 2)
                        nc.tensor.matmul(g_ps[:], wg_sb[:, pr, fc], n2_sb[:, pr, :],
                                         start=(e2 == 0), stop=(e2 == NE // 2 - 1),
                                         perf_mode=DR)
                    u_ps = s5ps.tile([P, QB], f32, tag="up", bufs=2)
                    for e2 in range(NE // 2):
                        pr = slice(2 * e2, 2 * e2 + 2)
                        nc.tensor.matmul(u_ps[:], wu_sb[:, pr, fc], n2_sb[:, pr, :],
                                         start=(e2 == 0), stop=(e2 == NE // 2 - 1),
                                         perf_mode=DR)
                    gel = s5t.tile([P, QB], f32, tag="gel", bufs=3)
                    nc.scalar.activation(gel[:], g_ps[:], AF.Gelu_apprx_tanh,
                                         scale=1.0 / 64.0)
                    nc.vector.tensor_mul(act8[:, f, :], gel[:], u_ps[:])
                return act8

            def emit_down(g, act8):
                order = (list(range(NE)) if g < 3 else
                         [e for e in range(NE) if e % 4 < 2]
                         + [e for e in range(NE) if e % 4 >= 2])
                for i, e in enumerate(order):
                    ec = slice(e * P, (e + 1) * P)
                    d_ps = s5ps.tile([P, QB], f32, tag="down", bufs=2)
                    for f2 in range(NF // 2):
                        pr = slice(2 * f2, 2 * f2 + 2)
                        nc.tensor.matmul(d_ps[:], wd_sb[:, pr, ec], act8[:, pr, :],
                                         start=(f2 == 0), stop=(f2 == NF // 2 - 1),
                                         perf_mode=DR)
                    d_sb = s5t.tile([P, QB], bf16, tag="dstage", bufs=3)
                    nc.scalar.activation(d_sb[:], d_ps[:], AF.Copy,
                                         scale=1.0 / 16384.0)
                    if g < 3:
                        dst = rs2_in[g][e * P : (e + 1) * P, :]
                    else:
                        t, j = e // 4, e % 4
                        row = t * 2 * P + (j % 2) * P
                        dst = rs2_in3[j // 2][row : row + P, :]
                    nc.sync.dma_start(r3(dst), d_sb[:])
                    if g == 3 and i == NE // 2 - 1:
                        nc.gpsimd.collective_compute(
                            "ReduceScatter", mybir.AluOpType.add,
                            replica_groups=GROUPS,
                            ins=[rs2_in3[0][:].opt()], outs=[rs2_out3[0][:].opt()],
                        )

            def emit_rs2(g):
                if g < 3:
                    i_ap, o_ap = rs2_in[g][:], rs2_out[g][:]
                else:
                    i_ap, o_ap = rs2_in3[1][:], rs2_out3[1][:]
                nc.gpsimd.collective_compute(
                    "ReduceScatter", mybir.AluOpType.add, replica_groups=GROUPS,
                    ins=[i_ap.opt()], outs=[o_ap.opt()],
                )

            def emit_stage6(g):
                cols = slice(g * QB, (g + 1) * QB)
                fsum = s5t.tile([P, H_LOC, QB], bf16, tag="fsum", bufs=2)
                if g < 3:
                    nc.sync.dma_start(fsum[:], r3(rs2_out[g]))
                else:
                    nc.sync.dma_start(fsum[:, 0:2, :], r3(rs2_out3[0]))
                    nc.sync.dma_start(fsum[:, 2:4, :], r3(rs2_out3[1]))
                fin = s5t.tile([P, H_LOC, QB], f32, tag="fin", bufs=2)
                nc.sync.dma_start(fin[:], r3(x2_scr, cols))
                nc.vector.tensor_add(fin[:], fin[:], fsum[:])
                nc.sync.dma_start(r3(out_ext, cols), fin[:])

            emit_rsqn2(2, s5t)           # x2q slot 0; AR(2) finished in stage 1
            act = emit_gateup(0)
            emit_stage3a(3, s5t, s5ps)   # waits RS1(3); overlaps group-0 compute
            emit_down(0, act)
            emit_rsqn2(3, s5t)           # AR(3) fired by stage3a(3)
            emit_ag2(1)                  # before RS2(0) so group 2 never waits
            emit_rs2(0)
            act = emit_gateup(1)
            emit_down(1, act)
            emit_stage6(0)
            emit_rs2(1)
            for g in (2, 3):
                act = emit_gateup(g)
                emit_down(g, act)
                emit_stage6(g - 1)
                emit_rs2(g)
            emit_stage6(3)


# ============================ host side ============================


def _prep_core_inputs(inputs):
    """Shard + transpose + fold rms scales into weights + quantize. 8 in_maps."""
    import ml_dtypes

    F8 = ml_dtypes.float8_e4m3
    BF = ml_dtypes.bfloat16

    x = np.asarray(inputs["x"], np.float32)          # [B, S, E]
    w_qkv = np.asarray(inputs["w_qkv"], np.float32)  # [E, H, 3D]
    w_out = np.asarray(inputs["w_out"], np.float32)  # [H, D, E]
    w_gate = np.asarray(inputs["w_gate"], np.float32)
    w_up = np.asarray(inputs["w_up"], np.float32)
    w_down = np.asarray(inputs["w_down"], np.float32)
    scale1 = np.asarray(inputs["scale1"], np.float32)
    scale2 = np.asarray(inputs["scale2"], np.float32)

    wqkv_s = w_qkv * scale1[:, None, None]
    wq_f = (wqkv_s[:, :, 0:HEAD_DIM] * 64.0).astype(F8)
    wk_f = (wqkv_s[:, :, HEAD_DIM : 2 * HEAD_DIM] * 64.0).astype(F8)
    wv_f = (wqkv_s[:, :, 2 * HEAD_DIM : 3 * HEAD_DIM] * 256.0).astype(F8)
    wout_f = (w_out.reshape(HEADS * HEAD_DIM, EMBED) * 64.0).astype(F8)
    wg_s = (w_gate * scale2[:, None] * 16.0).astype(F8)
    wu_s = (w_up * scale2[:, None] * 16.0).astype(F8)
    wd_s = (w_down * 256.0).astype(F8)

    kp = np.arange(P)[:, None]
    m = np.arange(QB + 3 * P)[None, :]
    masks = (m >= kp + 3 * P).astype(F8)  # mask_j = masks[:, (3-j)*128 : +512]

    in_maps = []
    for c in range(N_CORES):
        b, t = divmod(c, TP)
        hs = slice(H_LOC * t, H_LOC * (t + 1))
        fs = slice(F_LOC * t, F_LOC * (t + 1))
        xtb = np.ascontiguousarray(x[b].T)  # [E, S]
        in_maps.append(
            {
                "x8": (xtb * 4.0).astype(F8),
                "xte": np.ascontiguousarray(xtb[ROWS_T * t : ROWS_T * (t + 1), :]),
                "wq": np.ascontiguousarray(wq_f[:, hs, :]),
                "wk": np.ascontiguousarray(wk_f[:, hs, :]),
                "wv": np.ascontiguousarray(
                    wv_f[:, hs, :].reshape(EMBED, H_LOC * HEAD_DIM)),
                "wout": np.ascontiguousarray(
                    wout_f[H_LOC * HEAD_DIM * t : H_LOC * HEAD_DIM * (t + 1), :]
                ),
                "wg": np.ascontiguousarray(wg_s[:, fs]),
                "wu": np.ascontiguousarray(wu_s[:, fs]),
                "wd": np.ascontiguousarray(wd_s[fs, :]),
                "masks": np.ascontiguousarray(masks),
            }
        )
    return in_maps


def _install_profile_hook():
    import sys
    import types

    try:
        import antenv.axon_hooks  # noqa: F401

        return
    except ImportError:
        pass
    try:
        from trn_agent_boot.trn_boot import _ntff_profile_via_ctypes

        _hook = _ntff_profile_via_ctypes("/opt/axon/libaxon_pjrt.so")
        _mod = types.ModuleType("antenv.axon_hooks")
        _mod.get_axon_ntff_profile_hook = lambda: _hook
        sys.modules["antenv.axon_hooks"] = _mod
    except Exception:
        pass


def _run(nc, in_maps, trace=False, trace_cores=None):
    _install_profile_hook()
    from concourse.bass_utils import run_bass_kernel_spmd

    return run_bass_kernel_spmd(
        nc,
        in_maps,
        core_ids=list(range(N_CORES)),
        trace=trace,
        trace_cores=trace_cores,
    )


def kernel(**inputs):
    if "nc" not in _NC_CACHE:
        _NC_CACHE["nc"] = build_kernel()
    nc = _NC_CACHE["nc"]
    in_maps = _prep_core_inputs(inputs)
    res = _run(nc, in_maps)
    out = np.empty((BATCH, SEQ, EMBED), np.float32)
    for c in range(N_CORES):
        b, t = divmod(c, TP)
        out[b, :, ROWS_T * t : ROWS_T * (t + 1)] = res.results[c]["out"].T
    return out


if __name__ == "__main__":
    build_kernel()
    print("build ok")


# revision 41
# speedup vs baseline: 1.0621x; 1.0482x over previous
"""Trainium2 Bass kernel for a dense transformer block (RMSNorm->MHA->res, RMSNorm->SwiGLU-FFN->res).

Sharding over 8 NeuronCores: fsdp=2 (batch) x tp=4 (attention heads / FFN hidden).
Core 4*b + t handles batch b with TP-rank t (heads 4t..4t+3, FFN hidden cols 2048t..2048(t+1)).

All on-device activations are feature-major ("transposed": [features, rows]) so every
matmul (out = lhsT.T @ rhs, contraction on the partition axis) chains without transposes.

fp8 (e4m3) DoubleRow matmuls: contraction pairs two adjacent 128-row K-chunks per
instruction. Used for q/k/v projections, PV+softmax-denominator, out-projection, and
all three FFN matmuls. The logits matmul (contraction = head_dim = 128, unpairable)
runs in bf16.

RMSNorm1 is applied POST-projection: q = (x@W)/rms is a per-row (per moving column)
scalar multiply, so the q/k/v matmuls consume host-prequantized x8 = 4x directly and
the norm chain (squares -> ms -> rsqrt -> broadcast) runs concurrently on DVE/ACT;
only the cheap per-head PSUM->SBUF scale-muls wait for it.

Power-of-2 scale plan (host folds rms scales into weights):
  x8 = 4x  wq8/wk8 = 64*w  wv8 = 256*w -> q/k psum = 256*rms*q -> *bcq(=1/(256 rms)) bf16
  v8 = v_psum*bcq = 4*v
  expt8 = 32*exp(logits/sqrt(d))  [exp bias=ln32]   ones8 = 0.5 -> denom psum = 16*sum
  pv psum = 128*pv -> ao8 = pv_psum * (1/denom_psum) = 8*ao
  wout8 = 64*wout -> proj psum = 512*attn -> bf16 rs1 partials (x2^-9)
  x2 = xte + RS(bf16) [f32 residual stream]; x2n8 = 4*rms2(x2) -> fp8 AllGather
  wg8/wu8 = 16*w -> gate/up psum = 64*z -> gelu(scale 2^-6) f32; act8 = gel*u_psum = 64*act
  wd8 = 256*wd -> down psum = 16384*y -> bf16 rs2 partials (x2^-14)
Collectives: RS1/RS2 bf16, AllGather fp8, mean-square AllReduce f32 (tiny).

Stage 1+2 stream in 512-row phases. FFN runs in 4 row-groups of 512 with RS2(g)
pipelined under group g+1; the stage-1 tail (stage3a(3), rsqn2(2,3), AG2(1)) is
emitted inside the FFN scope so it overlaps FFN groups 0-1.
"""

import numpy as np

EMBED = 2048
HEADS = 16
HEAD_DIM = 128
FF_HID = 8192
BATCH = 2
SEQ = 2048
EPS = 1e-6

N_CORES = 8
TP = 4
GROUPS = [[0, 1, 2, 3], [4, 5, 6, 7]]
H_LOC = HEADS // TP          # 4 heads per core
F_LOC = FF_HID // TP         # 2048 ffn-hidden per core
ROWS = SEQ                   # 2048 rows per batch
ROWS_T = ROWS // TP          # 512 rows per tp-rank
P = 128
NE = EMBED // P              # 16 embed chunks
NF = F_LOC // P              # 16 ffn chunks
NR = ROWS // P               # 16 row chunks
QB = 512                     # q-block / phase row count / matmul moving size
NQB = ROWS // QB             # 4 phases
RH = 1024                    # ffn row-half (AllGather granularity)
INV_SQRT_D = float(1.0 / np.sqrt(HEAD_DIM))
LN32 = float(np.log(32.0))

_NC_CACHE = {}


def build_kernel():
    import concourse.mybir as mybir
    import concourse.tile as tile
    from concourse import bacc

    f32 = mybir.dt.float32
    bf16 = mybir.dt.bfloat16
    f8 = mybir.dt.float8e4

    nc = bacc.Bacc("TRN2", target_bir_lowering=False, debug=False, num_devices=N_CORES)

    io = {}
    io["x8"] = nc.dram_tensor("x8", [EMBED, ROWS], f8, kind="ExternalInput").ap()
    io["xte"] = nc.dram_tensor("xte", [ROWS_T, ROWS], f32, kind="ExternalInput").ap()
    io["wq"] = nc.dram_tensor("wq", [EMBED, H_LOC, HEAD_DIM], f8, kind="ExternalInput").ap()
    io["wk"] = nc.dram_tensor("wk", [EMBED, H_LOC, HEAD_DIM], f8, kind="ExternalInput").ap()
    io["wv"] = nc.dram_tensor("wv", [EMBED, H_LOC * HEAD_DIM], f8, kind="ExternalInput").ap()
    io["wout"] = nc.dram_tensor("wout", [H_LOC * HEAD_DIM, EMBED], f8, kind="ExternalInput").ap()
    io["wg"] = nc.dram_tensor("wg", [EMBED, F_LOC], f8, kind="ExternalInput").ap()
    io["wu"] = nc.dram_tensor("wu", [EMBED, F_LOC], f8, kind="ExternalInput").ap()
    io["wd"] = nc.dram_tensor("wd", [F_LOC, EMBED], f8, kind="ExternalInput").ap()
    io["masks"] = nc.dram_tensor("masks", [P, QB + 3 * P], f8, kind="ExternalInput").ap()
    io["out"] = nc.dram_tensor("out", [ROWS_T, ROWS], f32, kind="ExternalOutput").ap()

    with tile.TileContext(nc) as tc:
        _emit(tc, nc, io)
    nc.compile()
    return nc


def _emit(tc, nc, io):
    from contextlib import ExitStack

    import concourse.mybir as mybir

    f32 = mybir.dt.float32
    f32r = mybir.dt.float32r
    bf16 = mybir.dt.bfloat16
    f8 = mybir.dt.float8e4
    AF = mybir.ActivationFunctionType
    DR = mybir.MatmulPerfMode.DoubleRow

    x8in, xte = io["x8"], io["xte"]
    wq, wk, wv = io["wq"], io["wk"], io["wv"]
    wout, wg, wu, wd, masks = io["wout"], io["wg"], io["wu"], io["wd"], io["masks"]
    out_ext = io["out"]

    def r3(ap2d, cols=None):
        """[(o p), q] dram view -> [p, o, q]; optionally slice columns first."""
        v = ap2d if cols is None else ap2d[:, cols]
        return v.rearrange("(o p) q -> p o q", p=P)

    ctx = ExitStack()
    with ctx:
        consts = ctx.enter_context(tc.tile_pool(name="consts", bufs=1))
        dram = ctx.enter_context(tc.tile_pool(name="dram", bufs=1, space="DRAM"))
        # cross-scope pool: FFN gate/up weights + x2 tiles that span stage1->FFN
        xpool = ctx.enter_context(tc.tile_pool(name="xpool", bufs=1))

        # pair-axis stride of dual-fp8 Ldweights must be 16B-aligned -> pad cols
        ones8_t = consts.tile([P, 2, 16], f8)
        nc.vector.memset(ones8_t[:], 0.5)
        ones8 = ones8_t[:, :, 0:1]
        eps16_sb = consts.tile([1, 1], f32)
        nc.vector.memset(eps16_sb[:], EPS / 16.0)
        epsq_sb = consts.tile([1, 1], f32)
        nc.vector.memset(epsq_sb[:], EPS * 65536.0)
        ln32_sb = consts.tile([P, 1], f32)
        nc.vector.memset(ln32_sb[:], LN32)

        wg_sb = xpool.tile([P, NE, F_LOC], f8)
        wu_sb = xpool.tile([P, NE, F_LOC], f8)
        # two rotating x2 slots (phase qb uses slot qb%2), alive across scopes
        x2q_t = xpool.tile([P, 2, H_LOC, QB], f32)

        rs1_in = dram.tile([NQB, EMBED, ROWS_T], bf16)
        rs1_out = dram.tile([NQB, ROWS_T, ROWS_T], bf16)
        ar_in = dram.tile([NQB, 1, ROWS_T], f32)
        ar_out = dram.tile([NQB, 1, ROWS_T], f32)
        # one AllGather per 512-row group: group g's FFN input depends only on
        # rsqn2(g), so only group 3 sits on the late RS1(3)->AR(3) chain
        ag_in = dram.tile([NQB, ROWS_T, QB], f8)
        ag_out = dram.tile([NQB, EMBED, QB], f8)
        rs2_in = dram.tile([NQB, EMBED, ROWS_T], bf16)
        rs2_out = dram.tile([NQB, ROWS_T, ROWS_T], bf16)
        # group 3's RS2 split in E-halves so the tail collective is half-size:
        # half h holds e-chunks with (e%4)//2 == h, i.e. rank rows h*256..h*256+255
        rs2_in3 = dram.tile([2, EMBED // 2, ROWS_T], bf16)
        rs2_out3 = dram.tile([2, ROWS_T // 2, ROWS_T], bf16)
        x2_scr = dram.tile([ROWS_T, ROWS], f32)

        # ---- helpers shared by stage1 and FFN scopes ----
        def emit_stage3a(qb, pool, pspool):
            """x2 = rs1_out + xte slice; mean-square partials -> tiny AllReduce."""
            cols = slice(qb * QB, (qb + 1) * QB)
            rs_sb = pool.tile([P, H_LOC, QB], bf16, tag="rs_sb", bufs=1,
                              name=f"rs_sb{qb}")
            nc.sync.dma_start(rs_sb[:], r3(rs1_out[qb]))
            x2q = x2q_t[:, qb % 2]
            nc.sync.dma_start(x2q, r3(xte, cols))
            ms_part = pspool.tile([1, QB], f32, tag="acc1", bufs=2, name=f"msp{qb}")
            for em in range(H_LOC):
                nc.vector.tensor_add(x2q[:, em, :], x2q[:, em, :], rs_sb[:, em, :])
            for em2 in range(H_LOC // 2):
                pr = slice(2 * em2, 2 * em2 + 2)
                sq8 = pool.tile([P, 2, QB], f8, tag="sq", bufs=2)
                nc.vector.tensor_mul(sq8[:], x2q[:, pr, :], x2q[:, pr, :])
                nc.tensor.matmul(ms_part[:], ones8[:], sq8[:],
                                 start=(em2 == 0), stop=(em2 == H_LOC // 2 - 1),
                                 perf_mode=DR)
            nc.sync.dma_start(r3(x2_scr, cols), x2q)
            ms_sb = pool.tile([1, QB], f32, tag="ms_sb", bufs=1)
            nc.vector.tensor_copy(ms_sb[:], ms_part[:])
            nc.sync.dma_start(ar_in[qb][:], ms_sb[:])
            nc.gpsimd.collective_compute(
                "AllReduce", mybir.AluOpType.add, replica_groups=GROUPS,
                ins=[ar_in[qb][:].opt()], outs=[ar_out[qb][:].opt()],
            )

        def emit_rsqn2(qb, pool):
            """4/rms2 of the AllReduced mean-square, normalize to fp8, ship to AG."""
            arv = pool.tile([1, QB], f32, tag="arv", bufs=1)
            nc.sync.dma_start(arv[:], ar_out[qb][:])
            # ms_ar = 0.5*sum(x2^2) -> rms2/4 = sqrt(2*ms/(16E) + eps/16)
            rsq2 = pool.tile([1, QB], f32, tag="rsq2", bufs=1)
            nc.scalar.activation(rsq2[:], arv[:], AF.Sqrt, bias=eps16_sb[:],
                                 scale=1.0 / (8.0 * EMBED))
            rsq2_i = pool.tile([1, QB], f32, tag="rsq2i", bufs=1)
            nc.vector.reciprocal(rsq2_i[:], rsq2[:])
            bc2 = pool.tile([P, QB], f32, tag="bc", bufs=2)
            nc.gpsimd.partition_broadcast(bc2[:], rsq2_i[:])
            x2q = x2q_t[:, qb % 2]
            ag3 = r3(ag_in[qb])
            for em in range(H_LOC):
                n2q = pool.tile([P, QB], f8, tag="n2q", bufs=2)
                nc.vector.tensor_mul(n2q[:], x2q[:, em, :], bc2[:])
                nc.sync.dma_start(ag3[:, em, :], n2q[:])

        def emit_ag(g):
            nc.gpsimd.collective_compute(
                "AllGather", mybir.AluOpType.bypass, replica_groups=GROUPS,
                ins=[ag_in[g][:].opt()], outs=[ag_out[g][:].opt()],
            )

        # ========== Stage 1+2 (fused phases): qkv + attention (+rms2 prep) ==========
        with (
            tc.tile_pool(name="kv_store", bufs=1) as kv_pool,
            tc.tile_pool(name="s1", bufs=2) as s1,
            tc.tile_pool(name="s1ps", bufs=2, space="PSUM") as s1ps,
        ):
            k_store = kv_pool.tile([P, H_LOC, ROWS], bf16)
            v8_store = kv_pool.tile([P, NR, H_LOC, HEAD_DIM], f8)
            mask_sb = kv_pool.tile([P, QB + 3 * P], f8)
            wq_sb = kv_pool.tile([P, NE, H_LOC * HEAD_DIM], f8)
            wk_sb = kv_pool.tile([P, NE, H_LOC * HEAD_DIM], f8)
            wv_sb = kv_pool.tile([P, NE, H_LOC * HEAD_DIM], f8)
            wo_sb = kv_pool.tile([P, H_LOC, EMBED], f8)

            xns = {}

            def emit_x_dma(qb):
                cols = slice(qb * QB, (qb + 1) * QB)
                x8 = s1.tile([P, NE, QB], f8, tag="x8", bufs=2, name=f"x8_{qb}")
                nc.sync.dma_start(x8[:], r3(x8in, cols))
                xns[(qb, "8")] = x8

            # phase-0 critical DMAs first, then weights, then the FFN prefetch
            emit_x_dma(0)
            nc.sync.dma_start(wq_sb[:], wq.rearrange("(o p) h d -> p o (h d)", p=P))
            nc.sync.dma_start(wk_sb[:], wk.rearrange("(o p) h d -> p o (h d)", p=P))
            nc.sync.dma_start(wv_sb[:], r3(wv))
            nc.sync.dma_start(mask_sb[:], masks[:])
            nc.sync.dma_start(wo_sb[:], r3(wout))
            nc.sync.dma_start(wg_sb[:], r3(wg))
            nc.sync.dma_start(wu_sb[:], r3(wu))

            def emit_sq_ms_step(qb, e2):
                """square + mean-accumulate for chunk-pair e2 of phase qb (fp8 DR)."""
                if e2 == 0:
                    ms = s1ps.tile([1, QB], f32, tag="acc1", bufs=2, name=f"ms{qb}")
                    xns[(qb, "ms")] = ms
                ms = xns[(qb, "ms")]
                sq8 = s1.tile([P, 2, QB], f8, tag="sq", bufs=2)
                pr = slice(2 * e2, 2 * e2 + 2)
                sl = xns[(qb, "8")][:, pr, :]
                # (0.25*4x)^2 = x^2 (max ~28, no fp8 overflow)
                nc.scalar.activation(sq8[:], sl, AF.Square, scale=0.25)
                nc.tensor.matmul(ms[:], ones8[:], sq8[:],
                                 start=(e2 == 0), stop=(e2 == NE // 2 - 1),
                                 perf_mode=DR)

            def emit_norm_tail(qb):
                """bcq = 1/(256*rms): ms_psum = 8*sum(x^2) via fp8 squares of 4x."""
                ms = xns.pop((qb, "ms"))
                # ms_psum = 0.5*sum(x^2) -> 256*rms = sqrt(131072*ms/E + 65536*eps)
                rsq = s1.tile([1, QB], f32, tag="rsq", bufs=1)
                nc.scalar.activation(rsq[:], ms[:], AF.Sqrt, bias=epsq_sb[:],
                                     scale=131072.0 / EMBED)
                rsq_i = s1.tile([1, QB], f32, tag="rsqi", bufs=1)
                nc.vector.reciprocal(rsq_i[:], rsq[:])
                bc = s1.tile([P, QB], f32, tag="bc", bufs=2, name=f"bcq{qb}")
                nc.gpsimd.partition_broadcast(bc[:], rsq_i[:])
                xns[(qb, "bc")] = bc

            def emit_qkv_mm(qb):
                """q/k/v projections from host-quantized x8 (no norm dependency)."""
                x8 = xns[(qb, "8")]
                ps = {}
                for h in range(H_LOC):
                    hd = slice(h * HEAD_DIM, (h + 1) * HEAD_DIM)
                    for nm, w_sb in (("q", wq_sb), ("k", wk_sb)):
                        p_ps = s1ps.tile([P, QB], f32, tag="proj", bufs=2,
                                         name=f"{nm}ps{qb}_{h}")
                        for e2 in range(NE // 2):
                            pr = slice(2 * e2, 2 * e2 + 2)
                            nc.tensor.matmul(p_ps[:], w_sb[:, pr, hd], x8[:, pr, :],
                                             start=(e2 == 0),
                                             stop=(e2 == NE // 2 - 1), perf_mode=DR)
                        ps[(nm, h)] = p_ps
                v_ps = [
                    s1ps.tile([P, H_LOC * HEAD_DIM], f32, tag=t, bufs=2,
                              name=f"v_ps{i}")
                    for i, t in enumerate(("lg", "lg", "pv", "pv"))
                ]
                for e2 in range(NE // 2):
                    pr = slice(2 * e2, 2 * e2 + 2)
                    for rc in range(QB // P):
                        nc.tensor.matmul(v_ps[rc][:],
                                         x8[:, pr, rc * P : (rc + 1) * P],
                                         wv_sb[:, pr, :],
                                         start=(e2 == 0), stop=(e2 == NE // 2 - 1),
                                         perf_mode=DR)
                ps["v"] = v_ps
                return ps

            def emit_qkv_scale(qb, ps):
                """apply bcq per moving column; frees PSUM slots in FIFO order."""
                bc = xns.pop((qb, "bc"))
                cols = slice(qb * QB, (qb + 1) * QB)
                q_ph = s1.tile([P, H_LOC, QB], bf16, tag="q_ph", bufs=1,
                               name=f"q{qb}")
                for h in range(H_LOC):
                    nc.vector.tensor_mul(q_ph[:, h, :], ps[("q", h)][:], bc[:])
                    nc.vector.tensor_mul(k_store[:, h, cols], ps[("k", h)][:], bc[:])
                for rc in range(QB // P):
                    rcg = qb * (QB // P) + rc
                    nc.vector.tensor_mul(
                        v8_store[:, rcg].rearrange("p h d -> p (h d)"),
                        ps["v"][rc][:], bc[:])
                return q_ph

            def emit_attention(qb, q_ph):
                ao8 = s1.tile([P, H_LOC, QB], f8, tag="ao_ph", bufs=1, name=f"ao{qb}")
                nk = (qb + 1) * (QB // P)
                for h in range(H_LOC):
                    pv_ps = s1ps.tile([P, QB], f32, tag="pv", bufs=2)
                    sum_ps = s1ps.tile([1, QB], f32, tag="acc1", bufs=2)
                    lg_tiles = {}
                    ex_tiles = {}

                    def emit_lg(kc):
                        lg = s1ps.tile([P, QB], f32, tag="lg", bufs=2)
                        nc.tensor.matmul(
                            lg[:], k_store[:, h, kc * P : (kc + 1) * P],
                            q_ph[:, h, :], start=True, stop=True)
                        lg_tiles[kc] = lg

                    emit_lg(0)
                    for kc in range(nk):
                        if kc + 1 < nk:
                            emit_lg(kc + 1)
                        lg = lg_tiles.pop(kc)
                        if kc % 2 == 0:
                            ex = s1.tile([P, 2, QB], f8, tag="expt", bufs=2)
                            ex_tiles[kc // 2] = ex
                        ex = ex_tiles[kc // 2]
                        nc.scalar.activation(ex[:, kc % 2, :], lg[:], AF.Exp,
                                             bias=ln32_sb[:], scale=INV_SQRT_D)
                        j = kc - qb * (QB // P)
                        if j >= 0:
                            off = (3 - j) * P
                            nc.vector.tensor_mul(ex[:, kc % 2, :], ex[:, kc % 2, :],
                                                 mask_sb[:, off : off + QB])
                        if kc % 2 == 1:
                            pc = kc // 2
                            first, last = pc == 0, pc == nk // 2 - 1
                            nc.tensor.matmul(pv_ps[:],
                                             v8_store[:, 2 * pc : 2 * pc + 2, h, :],
                                             ex[:], start=first, stop=last,
                                             perf_mode=DR)
                            nc.tensor.matmul(sum_ps[:], ones8[:], ex[:],
                                             start=first, stop=last, perf_mode=DR)
                    rec = s1.tile([1, QB], f32, tag="rec", bufs=2)
                    nc.vector.reciprocal(rec[:], sum_ps[:])
                    rbc = s1.tile([P, QB], f32, tag="rbc", bufs=2)
                    nc.gpsimd.partition_broadcast(rbc[:], rec[:])
                    nc.vector.tensor_mul(ao8[:, h, :], pv_ps[:], rbc[:])
                return ao8

            def emit_outproj_step(qb, e, ao8):
                """one e-chunk of the out-projection partials of phase qb."""
                pr_ps = s1ps.tile([P, QB], f32, tag="proj", bufs=2)
                ec = slice(e * P, (e + 1) * P)
                for c2 in range(H_LOC // 2):
                    pr = slice(2 * c2, 2 * c2 + 2)
                    nc.tensor.matmul(pr_ps[:], wo_sb[:, pr, ec], ao8[:, pr, :],
                                     start=(c2 == 0), stop=(c2 == H_LOC // 2 - 1),
                                     perf_mode=DR)
                pr_sb = s1.tile([P, QB], bf16, tag="pr_sb", bufs=2)
                nc.scalar.activation(pr_sb[:], pr_ps[:], AF.Copy, scale=1.0 / 512.0)
                nc.sync.dma_start(
                    r3(rs1_in[qb][e * P : (e + 1) * P, :]), pr_sb[:])

            def emit_rs1(qb):
                nc.gpsimd.collective_compute(
                    "ReduceScatter", mybir.AluOpType.add, replica_groups=GROUPS,
                    ins=[rs1_in[qb][:].opt()], outs=[rs1_out[qb][:].opt()],
                )

            # ---- phase schedule (collectives pipelined under later phases) ----
            for e2 in range(NE // 2):
                emit_sq_ms_step(0, e2)
            emit_norm_tail(0)
            aos = {}
            for qb in range(NQB):
                ps = emit_qkv_mm(qb)
                q_ph = emit_qkv_scale(qb, ps)
                if qb + 1 < NQB:
                    emit_x_dma(qb + 1)
                aos[qb] = emit_attention(qb, q_ph)
                if qb >= 2:
                    emit_rsqn2(qb - 2, s1)
                    emit_ag(qb - 2)
                if qb + 1 < NQB:
                    for e in range(NE):
                        if e < NE // 2:
                            emit_sq_ms_step(qb + 1, e)
                        emit_outproj_step(qb, e, aos[qb])
                    emit_norm_tail(qb + 1)
                else:
                    for e in range(NE):
                        emit_outproj_step(qb, e, aos[qb])
                if qb >= 1:
                    emit_stage3a(qb - 1, s1, s1ps)
                emit_rs1(qb)

        # ========== Stage 5: FFN in 4 row-groups; stage-1 tail overlapped ==========
        with (
            tc.tile_pool(name="s5", bufs=1) as s5,
            tc.tile_pool(name="s5t", bufs=2) as s5t,
            tc.tile_pool(name="s5ps", bufs=2, space="PSUM") as s5ps,
        ):
            wd_sb = s5.tile([P, NF, EMBED], f8)

            def emit_gateup(g):
                n2_sb = s5t.tile([P, NE, QB], f8, tag="n2g", bufs=2)
                nc.sync.dma_start(n2_sb[:], r3(ag_out[g]))
                if g == 0:
                    nc.sync.dma_start(wd_sb[:], r3(wd))
                act8 = s5t.tile([P, NF, QB], f8, tag="act", bufs=2)
                for f in range(NF):
                    fc = slice(f * P, (f + 1) * P)
                    g_ps = s5ps.tile([P, QB], f32, tag="gate", bufs=2)
                    for e2 in range(NE // 2):
                        pr = slice(2 * e2, 2 * e2 + 2)
                        nc.tensor.matmul(g_ps[:], wg_sb[:, pr, fc], n2_sb[:, pr, :],
                                         start=(e2 == 0), stop=(e2 == NE // 2 - 1),
                                         perf_mode=DR)
                    u_ps = s5ps.tile([P, QB], f32, tag="up", bufs=2)
                    for e2 in range(NE // 2):
                        pr = slice(2 * e2, 2 * e2 + 2)
                        nc.tensor.matmul(u_ps[:], wu_sb[:, pr, fc], n2_sb[:, pr, :],
                                         start=(e2 == 0), stop=(e2 == NE // 2 - 1),
                                         perf_mode=DR)
                    gel = s5t.tile([P, QB], f32, tag="gel", bufs=3)
                    nc.scalar.activation(gel[:], g_ps[:], AF.Gelu_apprx_tanh,
                                         scale=1.0 / 64.0)
                    nc.vector.tensor_mul(act8[:, f, :], gel[:], u_ps[:])
                return act8

            def emit_down(g, act8):
                order = (list(range(NE)) if g < 3 else
                         [e for e in range(NE) if e % 4 < 2]
                         + [e for e in range(NE) if e % 4 >= 2])
                for i, e in enumerate(order):
                    ec = slice(e * P, (e + 1) * P)
                    d_ps = s5ps.tile([P, QB], f32, tag="down", bufs=2)
                    for f2 in range(NF // 2):
                        pr = slice(2 * f2, 2 * f2 + 2)
                        nc.tensor.matmul(d_ps[:], wd_sb[:, pr, ec], act8[:, pr, :],
                                         start=(f2 == 0), stop=(f2 == NF // 2 - 1),
                                         perf_mode=DR)
                    d_sb = s5t.tile([P, QB], bf16, tag="dstage", bufs=3)
                    nc.scalar.activation(d_sb[:], d_ps[:], AF.Copy,
                                         scale=1.0 / 16384.0)
                    if g < 3:
                        dst = rs2_in[g][e * P : (e + 1) * P, :]
                    else:
                        t, j = e // 4, e % 4
                        row = t * 2 * P + (j % 2) * P
                        dst = rs2_in3[j // 2][row : row + P, :]
                    nc.sync.dma_start(r3(dst), d_sb[:])
                    if g == 3 and i == NE // 2 - 1:
                        nc.gpsimd.collective_compute(
                            "ReduceScatter", mybir.AluOpType.add,
                            replica_groups=GROUPS,
                            ins=[rs2_in3[0][:].opt()], outs=[rs2_out3[0][:].opt()],
                        )

            def emit_rs2(g):
                if g < 3:
                    i_ap, o_ap = rs2_in[g][:], rs2_out[g][:]
                else:
                    i_ap, o_ap = rs2_in3[1][:], rs2_out3[1][:]
                nc.gpsimd.collective_compute(
                    "ReduceScatter", mybir.AluOpType.add, replica_groups=GROUPS,
                    ins=[i_ap.opt()], outs=[o_ap.opt()],
                )

            def emit_stage6(g):
                cols = slice(g * QB, (g + 1) * QB)
                fsum = s5t.tile([P, H_LOC, QB], bf16, tag="fsum", bufs=2)
                if g < 3:
                    nc.sync.dma_start(fsum[:], r3(rs2_out[g]))
                else:
                    nc.sync.dma_start(fsum[:, 0:2, :], r3(rs2_out3[0]))
                    nc.sync.dma_start(fsum[:, 2:4, :], r3(rs2_out3[1]))
                fin = s5t.tile([P, H_LOC, QB], f32, tag="fin", bufs=2)
                nc.sync.dma_start(fin[:], r3(x2_scr, cols))
                nc.vector.tensor_add(fin[:], fin[:], fsum[:])
                nc.sync.dma_start(r3(out_ext, cols), fin[:])

            emit_rsqn2(2, s5t)           # x2q slot 0; AR(2) finished in stage 1
            emit_ag(2)                   # group 2's input ready before group 1 ends
            act = emit_gateup(0)
            emit_stage3a(3, s5t, s5ps)   # waits RS1(3); overlaps group-0 compute
            emit_down(0, act)
            emit_rs2(0)
            emit_rsqn2(3, s5t)           # AR(3) fired by stage3a(3)
            emit_ag(3)                   # ~3 group-times of slack before group 3
            act = emit_gateup(1)
            emit_down(1, act)
            emit_stage6(0)
            emit_rs2(1)
            for g in (2, 3):
                act = emit_gateup(g)
                emit_down(g, act)
                emit_stage6(g - 1)
                emit_rs2(g)
            emit_stage6(3)


# ============================ host side ============================


def _prep_core_inputs(inputs):
    """Shard + transpose + fold rms scales into weights + quantize. 8 in_maps."""
    import ml_dtypes

    F8 = ml_dtypes.float8_e4m3
    BF = ml_dtypes.bfloat16

    x = np.asarray(inputs["x"], np.float32)          # [B, S, E]
    w_qkv = np.asarray(inputs["w_qkv"], np.float32)  # [E, H, 3D]
    w_out = np.asarray(inputs["w_out"], np.float32)  # [H, D, E]
    w_gate = np.asarray(inputs["w_gate"], np.float32)
    w_up = np.asarray(inputs["w_up"], np.float32)
    w_down = np.asarray(inputs["w_down"], np.float32)
    scale1 = np.asarray(inputs["scale1"], np.float32)
    scale2 = np.asarray(inputs["scale2"], np.float32)

    wqkv_s = w_qkv * scale1[:, None, None]
    wq_f = (wqkv_s[:, :, 0:HEAD_DIM] * 64.0).astype(F8)
    wk_f = (wqkv_s[:, :, HEAD_DIM : 2 * HEAD_DIM] * 64.0).astype(F8)
    wv_f = (wqkv_s[:, :, 2 * HEAD_DIM : 3 * HEAD_DIM] * 256.0).astype(F8)
    wout_f = (w_out.reshape(HEADS * HEAD_DIM, EMBED) * 64.0).astype(F8)
    wg_s = (w_gate * scale2[:, None] * 16.0).astype(F8)
    wu_s = (w_up * scale2[:, None] * 16.0).astype(F8)
    wd_s = (w_down * 256.0).astype(F8)

    kp = np.arange(P)[:, None]
    m = np.arange(QB + 3 * P)[None, :]
    masks = (m >= kp + 3 * P).astype(F8)  # mask_j = masks[:, (3-j)*128 : +512]

    in_maps = []
    for c in range(N_CORES):
        b, t = divmod(c, TP)
        hs = slice(H_LOC * t, H_LOC * (t + 1))
        fs = slice(F_LOC * t, F_LOC * (t + 1))
        xtb = np.ascontiguousarray(x[b].T)  # [E, S]
        in_maps.append(
            {
                "x8": (xtb * 4.0).astype(F8),
                "xte": np.ascontiguousarray(xtb[ROWS_T * t : ROWS_T * (t + 1), :]),
                "wq": np.ascontiguousarray(wq_f[:, hs, :]),
                "wk": np.ascontiguousarray(wk_f[:, hs, :]),
                "wv": np.ascontiguousarray(
                    wv_f[:, hs, :].reshape(EMBED, H_LOC * HEAD_DIM)),
                "wout": np.ascontiguousarray(
                    wout_f[H_LOC * HEAD_DIM * t : H_LOC * HEAD_DIM * (t + 1), :]
                ),
                "wg": np.ascontiguousarray(wg_s[:, fs]),
                "wu": np.ascontiguousarray(wu_s[:, fs]),
                "wd": np.ascontiguousarray(wd_s[fs, :]),
                "masks": np.ascontiguousarray(masks),
            }
        )
    return in_maps


def _install_profile_hook():
    import sys
    import types

    try:
        import antenv.axon_hooks  # noqa: F401

        return
    except ImportError:
        pass
    try:
        from trn_agent_boot.trn_boot import _ntff_profile_via_ctypes

        _hook = _ntff_profile_via_ctypes("/opt/axon/libaxon_pjrt.so")
        _mod = types.ModuleType("antenv.axon_hooks")
        _mod.get_axon_ntff_profile_hook = lambda: _hook
        sys.modules["antenv.axon_hooks"] = _mod
    except Exception:
        pass


def _run(nc, in_maps, trace=False, trace_cores=None):
    _install_profile_hook()
    from concourse.bass_utils import run_bass_kernel_spmd

    return run_bass_kernel_spmd(
        nc,
        in_maps,
        core_ids=list(range(N_CORES)),
        trace=trace,
        trace_cores=trace_cores,
    )


def kernel(**inputs):
    if "nc" not in _NC_CACHE:
        _NC_CACHE["nc"] = build_kernel()
    nc = _NC_CACHE["nc"]
    in_maps = _prep_core_inputs(inputs)
    res = _run(nc, in_maps)
    out = np.empty((BATCH, SEQ, EMBED), np.float32)
    for c in range(N_CORES):
        b, t = divmod(c, TP)
        out[b, :, ROWS_T * t : ROWS_T * (t + 1)] = res.results[c]["out"].T
    return out


if __name__ == "__main__":
    build_kernel()
    print("build ok")
